# revision 18
# baseline (speedup 1.0000x reference)
"""GCN block (BN -> lrelu -> GCNConv -> lrelu -> BN -> lrelu -> GCNConv -> lrelu)
on 8 Trainium2 NeuronCores via Bass/Tile.

Strategy: shard nodes (destinations) across cores; bucket edges by destination
window (64 dests) on the host; aggregate per window with one-hot fp16 matmuls
accumulating in fp32 PSUM; gather source features with dma_gather from an
AllGathered fp16 replica of h' = dinv * h; self-loops folded in analytically
in fp32. deg via the same one-hot matmuls against a ones vector. BN statistics
via matmul-with-ones partials + a tiny AllReduce.
"""

import os
import time

import numpy as np

EPS = 1e-5
SLOPE = 0.1
WIN = 64  # dests per window
GB = 16  # blocks per gather batch
AB = 16  # blocks per A-build batch

_timing = {}


class _StopBuild(Exception):
    pass


def _host_prep(edge_index, edge_attr, n_nodes, n_cores):
    """Bucket edges by (core, window); build uniform per-core slot arrays."""
    row = np.asarray(edge_index[0], dtype=np.int64)
    col = np.asarray(edge_index[1], dtype=np.int64)
    w = np.asarray(edge_attr[:, 0], dtype=np.float32)

    nloc = n_nodes // n_cores
    nwin = nloc // WIN

    order = np.argsort(col, kind="stable")
    row, col, w = row[order], col[order], w[order]

    gwin = col // WIN  # global window id: 0 .. n_cores*nwin-1
    counts = np.bincount(gwin, minlength=n_cores * nwin)
    K = int(np.ceil(counts.max() / 128)) if counts.max() > 0 else 1
    S = K * 128  # slots per window
    nblk = nwin * K  # blocks per core
    P = nwin * S  # slots per core

    offs = np.concatenate([[0], np.cumsum(counts)])
    rank = np.arange(len(row)) - offs[gwin]
    win_local = gwin % nwin
    core_of = gwin // nwin
    slot = win_local * S + rank  # within-core slot

    cores = []
    for ci in range(n_cores):
        m = core_of == ci
        er, ec, ew, es = row[m], col[m], w[m], slot[m]
        srow = np.zeros(P, dtype=np.int64)
        scol = np.full(P, 999.0, dtype=np.float16)  # sentinel: no match
        sw = np.zeros(P, dtype=np.float32)
        srow[es] = er
        scol[es] = (ec % WIN).astype(np.float16)
        sw[es] = ew
        parity = (srow & 1).astype(np.float32)
        pair = (srow >> 1).astype(np.int16)

        idx16 = np.ascontiguousarray(pair.reshape(P // 16, 16).T)  # [16, P/16]
        idx16 = np.tile(idx16, (8, 1))  # [128, P/16]
        mcol = np.ascontiguousarray(scol.reshape(nblk, 128).T)  # [128, nblk]
        swe = (sw * (1.0 - parity)).astype(np.float16)
        swo = (sw * parity).astype(np.float16)
        mwe = np.ascontiguousarray(swe.reshape(nblk, 128).T)
        mwo = np.ascontiguousarray(swo.reshape(nblk, 128).T)

        def dbl(a):  # [128, nblk] -> [128, nblk, 2] duplicated values
            return np.ascontiguousarray(np.repeat(a[:, :, None], 2, axis=2))

        mw = np.ascontiguousarray(sw.astype(np.float16).reshape(nblk, 128).T)
        cores.append(
            dict(idx16=idx16, mcol=dbl(mcol), mwe=dbl(mwe), mwo=dbl(mwo), mw=mw)
        )

    return cores, K, nwin, nblk, P, nloc


def _build_nc(n_nodes, n_cores, K, nwin, nblk, P, nloc, stop_after="all", no_cc=False):
    import concourse.bacc as bacc
    import concourse.mybir as mybir
    import concourse.tile as tile
    from concourse.library_config import mlp

    f32 = mybir.dt.float32
    f16 = mybir.dt.float16
    Alu = mybir.AluOpType

    npair = n_nodes // 2
    nch = nloc // 128  # node chunks per core (= window pairs)
    npr = nwin // 2
    assert npr == nch

    nc = bacc.Bacc(
        "TRN2", target_bir_lowering=False, debug=False, num_devices=n_cores
    )

    # --- I/O ---
    x = nc.dram_tensor("x", [nloc, 64], f32, kind="ExternalInput")
    w1 = nc.dram_tensor("w1", [64, 64], f32, kind="ExternalInput")
    w2 = nc.dram_tensor("w2", [64, 64], f32, kind="ExternalInput")
    bn1g = nc.dram_tensor("bn1g", [64, 1], f32, kind="ExternalInput")
    bn1b = nc.dram_tensor("bn1b", [64, 1], f32, kind="ExternalInput")
    bn2g = nc.dram_tensor("bn2g", [64, 1], f32, kind="ExternalInput")
    bn2b = nc.dram_tensor("bn2b", [64, 1], f32, kind="ExternalInput")
    b1r = nc.dram_tensor("b1r", [128, 64], f32, kind="ExternalInput")
    b2r = nc.dram_tensor("b2r", [128, 64], f32, kind="ExternalInput")
    idxd = nc.dram_tensor("idxd", [128, P // 16], mybir.dt.int16, kind="ExternalInput")
    iotad = nc.dram_tensor("iotad", [128, WIN], f16, kind="ExternalInput")
    id128d = nc.dram_tensor("id128d", [128, 128], f32, kind="ExternalInput")
    id64d = nc.dram_tensor("id64d", [64, 64], f32, kind="ExternalInput")
    mcold = nc.dram_tensor("mcold", [128, nblk, 2], f16, kind="ExternalInput")
    mwed = nc.dram_tensor("mwed", [128, nblk, 2], f16, kind="ExternalInput")
    mwod = nc.dram_tensor("mwod", [128, nblk, 2], f16, kind="ExternalInput")
    mwd = nc.dram_tensor("mwd", [128, nblk], f16, kind="ExternalInput")
    out = nc.dram_tensor("out", [nloc, 64], f32, kind="ExternalOutput")

    groups = [list(range(n_cores))]

    with tile.TileContext(nc) as tc:
      try:
        with (
            tc.tile_pool(name="pc", bufs=1) as pc,
            tc.tile_pool(name="pbig", bufs=1) as pbig,
            tc.tile_pool(name="pw", bufs=4) as pw,
            tc.tile_pool(name="pg", bufs=2) as pg,
            tc.tile_pool(name="ps", bufs=1, space="PSUM") as pstat,
            tc.tile_pool(name="pp", bufs=3, space="PSUM") as pagg,
            tc.tile_pool(name="pm", bufs=2, space="PSUM") as pmm,
            tc.tile_pool(name="pd", bufs=1, space="PSUM") as pdeg,
            tc.tile_pool(name="dram", bufs=1, space="DRAM") as dram,
        ):
            nc.gpsimd.load_library(mlp)

            # ---------- constants & loads ----------
            idx_t = pc.tile([128, P // 16], mybir.dt.int16)
            nc.sync.dma_start(idx_t[:], idxd[:])
            mcol_t = pc.tile([128, nblk, 2], f16)
            nc.sync.dma_start(mcol_t[:], mcold[:])
            mwe_t = pc.tile([128, nblk, 2], f16)
            nc.sync.dma_start(mwe_t[:], mwed[:])
            mwo_t = pc.tile([128, nblk, 2], f16)
            nc.sync.dma_start(mwo_t[:], mwod[:])
            mw_t = pc.tile([128, nblk], f16)
            nc.sync.dma_start(mw_t[:], mwd[:])

            w1_t = pc.tile([64, 64], f32)
            nc.sync.dma_start(w1_t[:], w1[:])
            w2_t = pc.tile([64, 64], f32)
            nc.sync.dma_start(w2_t[:], w2[:])
            bn1g_t = pc.tile([64, 1], f32)
            nc.sync.dma_start(bn1g_t[:], bn1g[:])
            bn1b_t = pc.tile([64, 1], f32)
            nc.sync.dma_start(bn1b_t[:], bn1b[:])
            bn2g_t = pc.tile([64, 1], f32)
            nc.sync.dma_start(bn2g_t[:], bn2g[:])
            bn2b_t = pc.tile([64, 1], f32)
            nc.sync.dma_start(bn2b_t[:], bn2b[:])
            b1r_t = pc.tile([128, 1, 64], f32)
            nc.sync.dma_start(b1r_t[:, 0, :], b1r[:])
            b2r_t = pc.tile([128, 1, 64], f32)
            nc.sync.dma_start(b2r_t[:, 0, :], b2r[:])

            # big_a: x chunks early, reused as final z2 late
            big_a = pbig.tile([128, nch, 64], f32, tag="big_a")
            nc.sync.dma_start(big_a[:], x.rearrange("(c p) f -> p c f", p=128))

            iota_f = pc.tile([128, 1, WIN], f16)
            nc.sync.dma_start(iota_f[:, 0, :], iotad[:])
            id128 = pc.tile([128, 128], f32)
            nc.sync.dma_start(id128[:], id128d[:])
            id64 = pc.tile([64, 64], f32)
            nc.sync.dma_start(id64[:], id64d[:])
            ones16 = pc.tile([128, 1], f16)
            nc.vector.memset(ones16[:], 1.0)
            ones32 = pc.tile([128, 1], f32)
            nc.vector.memset(ones32[:], 1.0)

            # DRAM intermediates
            h1_full = dram.tile([npair, 128], f16, tag="h1f")
            h2_full = dram.tile([npair, 128], f16, tag="h2f")
            h1_b = dram.tile([nloc, 64], f16, tag="h1b")
            h2_b = dram.tile([nloc, 64], f16, tag="h2b")

            # ---------- helper: A-build for a batch of blocks ----------
            def build_A(b0, nb):
                # [128, nb, 32, 2] views: innermost step-1 pair over doubled
                # metadata keeps every AP 2x_1p-eligible on DVE (fp16 2x mode)
                g2 = WIN // 2
                eq = pw.tile([128, AB, g2, 2], f16, tag="eq")
                iot = iota_f[:, :, :].rearrange("p o (g t) -> p o g t", t=2)
                nc.vector.tensor_tensor(
                    out=eq[:, 0:nb, :, :],
                    in0=iot.to_broadcast([128, nb, g2, 2]),
                    in1=mcol_t[:, b0 : b0 + nb, None, :].to_broadcast(
                        [128, nb, g2, 2]
                    ),
                    op=Alu.is_equal,
                )
                ae = pw.tile([128, AB, g2, 2], f16, tag="ae")
                nc.vector.tensor_tensor(
                    out=ae[:, 0:nb, :, :],
                    in0=eq[:, 0:nb, :, :],
                    in1=mwe_t[:, b0 : b0 + nb, None, :].to_broadcast(
                        [128, nb, g2, 2]
                    ),
                    op=Alu.mult,
                )
                ao = pw.tile([128, AB, g2, 2], f16, tag="ao")
                nc.vector.tensor_tensor(
                    out=ao[:, 0:nb, :, :],
                    in0=eq[:, 0:nb, :, :],
                    in1=mwo_t[:, b0 : b0 + nb, None, :].to_broadcast(
                        [128, nb, g2, 2]
                    ),
                    op=Alu.mult,
                )
                return ae, ao

            # ---------- deg pass: deg = 1 + sum_w by dest (eq-only A, w as rhs)
            def build_eq(b0, nb):
                g2 = WIN // 2
                eq = pw.tile([128, AB, g2, 2], f16, tag="eq")
                iot = iota_f[:, :, :].rearrange("p o (g t) -> p o g t", t=2)
                nc.vector.tensor_tensor(
                    out=eq[:, 0:nb, :, :],
                    in0=iot.to_broadcast([128, nb, g2, 2]),
                    in1=mcol_t[:, b0 : b0 + nb, None, :].to_broadcast(
                        [128, nb, g2, 2]
                    ),
                    op=Alu.is_equal,
                )
                return eq

            deg_t = pc.tile([128, nch], f32)
            eq_d = None
            for j in range(npr):  # window pair j -> dests [128j, 128j+128)
                dps = pdeg.tile([128, 1], f32, tag="deg")
                for half in range(2):
                    wnd = 2 * j + half
                    for k in range(K):
                        b = wnd * K + k
                        if b % AB == 0:
                            eq_d = build_eq(b, min(AB, nblk - b))
                        jj = b % AB
                        nc.tensor.matmul(
                            dps[64 * half : 64 * half + 64, :],
                            lhsT=eq_d[:, jj, :, :],
                            rhs=mw_t[:, b : b + 1],
                            start=(k == 0),
                            stop=(k == K - 1),
                        )
                nc.vector.tensor_scalar(
                    deg_t[:, j : j + 1], dps[:], 1.0, None, Alu.add
                )

            # dinv = rsqrt(deg): sqrt + reciprocal + one Newton step
            sqd_t = pw.tile([128, nch], f32, tag="nwt_s")
            nc.scalar.sqrt(sqd_t[:], deg_t[:])
            r0_t = pw.tile([128, nch], f32, tag="nwt_r")
            nc.vector.reciprocal(r0_t[:], sqd_t[:])
            a_t = pw.tile([128, nch], f32, tag="nwt_a")
            nc.vector.tensor_tensor(a_t[:], r0_t[:], r0_t[:], op=Alu.mult)
            nc.vector.tensor_tensor(a_t[:], deg_t[:], a_t[:], op=Alu.mult)
            nc.vector.tensor_scalar(a_t[:], a_t[:], -0.5, 1.5, Alu.mult, Alu.add)
            dinv_t = pc.tile([128, nch], f32)
            nc.vector.tensor_tensor(dinv_t[:], r0_t[:], a_t[:], op=Alu.mult)
            dinv2_t = pc.tile([128, nch], f32)
            nc.vector.tensor_tensor(dinv2_t[:], dinv_t[:], dinv_t[:], op=Alu.mult)

            # ---------- BN stats helper (node-major input chunks) ----------
            def bn_stats(src_all, tag):
                sps = pstat.tile([64, 1], f32, tag="ssum")
                spq = pstat.tile([64, 64], f32, tag="sq")
                for c in range(nch):
                    nc.tensor.matmul(
                        sps[:], lhsT=src_all[:, c, :], rhs=ones32[:],
                        start=(c == 0), stop=(c == nch - 1),
                    )
                for c in range(nch):
                    nc.tensor.matmul(
                        spq[:], lhsT=src_all[:, c, :], rhs=src_all[:, c, :],
                        start=(c == 0), stop=(c == nch - 1),
                    )
                st = pw.tile([64, 2], f32, tag="st")
                nc.vector.tensor_copy(st[:, 0:1], sps[:])
                dq = pw.tile([64, 64], f32, tag="dq")
                nc.vector.tensor_tensor(dq[:], spq[:], id64[:], op=Alu.mult)
                nc.vector.reduce_sum(st[:, 1:2], dq[:], axis=mybir.AxisListType.X)
                sin = dram.tile([64, 2], f32, tag=tag + "_in")
                sout = dram.tile([64, 2], f32, tag=tag + "_out")
                nc.gpsimd.dma_start(sin[:], st[:])
                if no_cc:
                    nc.gpsimd.dma_start(sout[:], sin[:])
                else:
                    nc.gpsimd.collective_compute(
                        "AllReduce", Alu.add, replica_groups=groups,
                        ins=[sin.opt()], outs=[sout.opt()],
                    )
                sr = pw.tile([64, 2], f32, tag="str")
                nc.gpsimd.dma_start(sr[:], sout[:])
                return sr

            def bn_coeffs(sr, g_t, b_t, tag):
                m = pw.tile([64, 1], f32, tag=tag + "m")
                nc.vector.tensor_scalar(m[:], sr[:, 0:1], 1.0 / n_nodes, None, Alu.mult)
                ms = pw.tile([64, 1], f32, tag=tag + "ms")
                nc.vector.tensor_scalar(ms[:], sr[:, 1:2], 1.0 / n_nodes, None, Alu.mult)
                v = pw.tile([64, 1], f32, tag=tag + "v")
                nc.vector.tensor_tensor(v[:], m[:], m[:], op=Alu.mult)
                nc.vector.tensor_tensor(v[:], ms[:], v[:], op=Alu.subtract)
                nc.vector.tensor_scalar(v[:], v[:], EPS, None, Alu.add)
                s = pw.tile([64, 1], f32, tag=tag + "s")
                nc.scalar.sqrt(s[:], v[:])
                r0 = pw.tile([64, 1], f32, tag=tag + "r0")
                nc.vector.reciprocal(r0[:], s[:])
                aa = pw.tile([64, 1], f32, tag=tag + "aa")
                nc.vector.tensor_tensor(aa[:], r0[:], r0[:], op=Alu.mult)
                nc.vector.tensor_tensor(aa[:], v[:], aa[:], op=Alu.mult)
                nc.vector.tensor_scalar(aa[:], aa[:], -0.5, 1.5, Alu.mult, Alu.add)
                r1 = pw.tile([64, 1], f32, tag=tag + "r1")
                nc.vector.tensor_tensor(r1[:], r0[:], aa[:], op=Alu.mult)
                scl = pw.tile([64, 1], f32, tag=tag + "scl")
                nc.vector.tensor_tensor(scl[:], r1[:], g_t[:], op=Alu.mult)
                sh = pw.tile([64, 1], f32, tag=tag + "sh")
                nc.vector.tensor_tensor(sh[:], m[:], scl[:], op=Alu.mult)
                nc.vector.tensor_tensor(sh[:], b_t[:], sh[:], op=Alu.subtract)
                return scl, sh

            # ---------- feature-major block: transpose, BN apply, lrelu, @W ----------
            def feat_block(src_all, scl, sh, w_t, hp_all, self_all):
                ft = pbig.tile([64, nloc], f32, tag="featT")
                for c in range(nch):
                    tp = pmm.tile([64, 128], f32, tag="mm")
                    nc.tensor.transpose(tp[:], src_all[:, c, :], id128[:])
                    nc.vector.tensor_copy(ft[:, 128 * c : 128 * (c + 1)], tp[:])
                nc.vector.tensor_scalar(ft[:], ft[:], scl[:], sh[:], Alu.mult, Alu.add)
                for c in range(nch):
                    sl = ft[:, 128 * c : 128 * (c + 1)]
                    lt = pw.tile([64, 128], f32, tag="lrT")
                    nc.scalar.mul(lt[:], sl, SLOPE)
                    nc.vector.tensor_tensor(sl, sl, lt[:], op=Alu.max)
                    hp = pmm.tile([128, 64], f32, tag="mm")
                    nc.tensor.matmul(hp[:], lhsT=sl, rhs=w_t[:], start=True, stop=True)
                    nc.scalar.activation(
                        hp_all[:, c, :], hp[:],
                        mybir.ActivationFunctionType.Copy,
                        bias=0.0, scale=dinv_t[:, c : c + 1],
                    )
                    nc.vector.tensor_scalar(
                        self_all[:, c, :], hp[:], dinv2_t[:, c : c + 1],
                        None, Alu.mult,
                    )

            if stop_after == "deg":
                nc.sync.dma_start(
                    out.rearrange("(c p) k -> p c k", p=128)[:, :, 0:1],
                    dinv_t[:, :, None],
                )
                raise _StopBuild
            # ---------- BN1 on x, h1' ----------
            sr1 = bn_stats(big_a, "st1")
            scl1, sh1 = bn_coeffs(sr1, bn1g_t, bn1b_t, "bn1")
            hp_all = pbig.tile([128, nch, 64], f16, tag="hp")
            self1 = pbig.tile([128, nch, 64], f32, tag="self1")
            feat_block(big_a, scl1, sh1, w1_t, hp_all, self1)
            if stop_after == "h1":
                nc.sync.dma_start(out.rearrange("(c p) f -> p c f", p=128), self1[:])
                raise _StopBuild
            nc.gpsimd.dma_start(h1_b.rearrange("(c p) f -> p c f", p=128), hp_all[:])
            if no_cc:
                for ci in range(n_cores):
                    nc.gpsimd.dma_start(
                        h1_full[ci * (nloc // 2) : (ci + 1) * (nloc // 2), :],
                        h1_b.rearrange("(a b) c -> a (b c)", b=2),
                    )
            else:
                nc.gpsimd.collective_compute(
                    "AllGather", Alu.bypass, replica_groups=groups,
                    ins=[h1_b.opt()], outs=[h1_full.opt()],
                )

            # ---------- aggregation pass ----------
            def agg_pass(h_full, z_all):
                ngb = (nblk + GB - 1) // GB
                ps = None
                ae_t = ao_t = None
                for g in range(ngb):
                    b0 = g * GB
                    nb = min(GB, nblk - b0)
                    mt = pg.tile([128, GB, 128], f16, tag="m")
                    nc.gpsimd.dma_gather(
                        out_ap=mt[:, 0:nb, :],
                        in_ap=h_full[:],
                        idxs_ap=idx_t[:, b0 * 8 : (b0 + nb) * 8],
                        num_idxs=nb * 128,
                        num_idxs_reg=nb * 128,
                        elem_size=128,
                        single_packet=False,
                    )
                    for b in range(b0, b0 + nb):
                        if b % AB == 0:
                            ae_t, ao_t = build_A(b, min(AB, nblk - b))
                        jj = b % AB
                        wnd = b // K
                        k = b % K
                        j = wnd // 2
                        half = wnd % 2
                        if half == 0 and k == 0:
                            ps = pagg.tile([128, 64], f32, tag="agg")
                        mj = b - b0
                        nc.tensor.matmul(
                            ps[64 * half : 64 * half + 64, :],
                            lhsT=ae_t[:, jj, :, :],
                            rhs=mt[:, mj, 0:64],
                            start=(k == 0),
                            stop=False,
                        )
                        nc.tensor.matmul(
                            ps[64 * half : 64 * half + 64, :],
                            lhsT=ao_t[:, jj, :, :],
                            rhs=mt[:, mj, 64:128],
                            start=False,
                            stop=(k == K - 1),
                        )
                        if half == 1 and k == K - 1:
                            nc.scalar.activation(
                                z_all[:, j, :], ps[:],
                                mybir.ActivationFunctionType.Copy,
                                bias=0.0, scale=dinv_t[:, j : j + 1],
                            )

            def post_agg(z_all, self_all, br_t):
                nc.vector.tensor_tensor(z_all[:], z_all[:], self_all[:], op=Alu.add)
                nc.vector.tensor_tensor(
                    z_all[:], z_all[:], br_t[:].to_broadcast([128, nch, 64]),
                    op=Alu.add,
                )
                for c in range(nch):
                    sl = z_all[:, c, :]
                    lt = pw.tile([128, 64], f32, tag="lrz")
                    nc.scalar.mul(lt[:], sl, SLOPE)
                    nc.vector.tensor_tensor(sl, sl, lt[:], op=Alu.max)

            if stop_after == "ag1":
                nc.sync.dma_start(out.rearrange("(c p) f -> p c f", p=128), self1[:])
                raise _StopBuild
            if stop_after == "aggnc":
                # fill h1_full locally (replicated shards) to test agg without AllGather
                for ci in range(n_cores):
                    nc.gpsimd.dma_start(
                        h1_full[ci * (nloc // 2) : (ci + 1) * (nloc // 2), :],
                        h1_b.rearrange("(a b) c -> a (b c)", b=2),
                    )
                z_all = pbig.tile([128, nch, 64], f32, tag="z_all")
                agg_pass(h1_full, z_all)
                nc.sync.dma_start(out.rearrange("(c p) f -> p c f", p=128), z_all[:])
                raise _StopBuild
            z_all = pbig.tile([128, nch, 64], f32, tag="z_all")
            agg_pass(h1_full, z_all)
            post_agg(z_all, self1, b1r_t)
            if stop_after == "agg1":
                nc.sync.dma_start(out.rearrange("(c p) f -> p c f", p=128), z_all[:])
                raise _StopBuild

            # ---------- BN2, h2' ----------
            sr2 = bn_stats(z_all, "st2")
            scl2, sh2 = bn_coeffs(sr2, bn2g_t, bn2b_t, "bn2")
            hp2_all = pbig.tile([128, nch, 64], f16, tag="hp")
            self2 = pbig.tile([128, nch, 64], f32, tag="self1")
            feat_block(z_all, scl2, sh2, w2_t, hp2_all, self2)
            if stop_after == "bn2":
                nc.sync.dma_start(out.rearrange("(c p) f -> p c f", p=128), self2[:])
                raise _StopBuild
            nc.gpsimd.dma_start(h2_b.rearrange("(c p) f -> p c f", p=128), hp2_all[:])
            if no_cc:
                for ci in range(n_cores):
                    nc.gpsimd.dma_start(
                        h2_full[ci * (nloc // 2) : (ci + 1) * (nloc // 2), :],
                        h2_b.rearrange("(a b) c -> a (b c)", b=2),
                    )
            else:
                nc.gpsimd.collective_compute(
                    "AllGather", Alu.bypass, replica_groups=groups,
                    ins=[h2_b.opt()], outs=[h2_full.opt()],
                )

            # ---------- layer 2 aggregation + final ----------
            z2_all = pbig.tile([128, nch, 64], f32, tag="big_a")
            agg_pass(h2_full, z2_all)
            post_agg(z2_all, self2, b2r_t)
            nc.sync.dma_start(out.rearrange("(c p) f -> p c f", p=128), z2_all[:])
      except _StopBuild:
        pass

    nc.compile()
    return nc


def _run_spmd(nc, in_maps, n_cores, timeit=0):
    """Run via PJRT/shard_map (axon path), optionally timing repeated execs."""
    import jax
    from jax.sharding import Mesh, NamedSharding, PartitionSpec
    from jax.experimental.shard_map import shard_map
    import concourse.mybir as mybir
    from concourse import bass2jax

    bass2jax.install_neuronx_cc_hook()

    partition_name = nc.partition_id_tensor.name if nc.partition_id_tensor else None
    in_names, out_names, out_avals, zero_outs = [], [], [], []
    for alloc in nc.m.functions[0].allocations:
        if not isinstance(alloc, mybir.MemoryLocationSet):
            continue
        name = alloc.memorylocations[0].name
        if alloc.kind == "ExternalInput":
            if name != partition_name:
                in_names.append(name)
        elif alloc.kind == "ExternalOutput":
            shape = tuple(alloc.tensor_shape)
            dtype = mybir.dt.np(alloc.dtype)
            out_names.append(name)
            out_avals.append(jax.core.ShapedArray(shape, dtype))
            zero_outs.append(np.zeros(shape, dtype))
    n_params = len(in_names)
    all_in_names = in_names + out_names + ([partition_name] if partition_name else [])

    def _body(*args):
        operands = list(args)
        if partition_name is not None:
            operands.append(bass2jax.partition_id_tensor())
        outs = bass2jax._bass_exec_p.bind(
            *operands,
            out_avals=tuple(out_avals),
            in_names=tuple(all_in_names),
            out_names=tuple(out_names),
            lowering_input_output_aliases=(),
            sim_require_finite=True,
            sim_require_nnan=True,
            nc=nc,
        )
        return tuple(outs)

    devices = jax.devices()[:n_cores]
    mesh = Mesh(np.asarray(devices), ("core",))
    in_specs = (PartitionSpec("core"),) * (n_params + len(out_names))
    out_specs = (PartitionSpec("core"),) * len(out_names)
    sharded = jax.jit(
        shard_map(_body, mesh=mesh, in_specs=in_specs, out_specs=out_specs,
                  check_rep=False),
        keep_unused=True,
    )
    concat_in = [
        np.concatenate([np.asarray(in_maps[c][nm]) for c in range(n_cores)], axis=0)
        for nm in in_names
    ]
    concat_zeros = [
        np.zeros((n_cores * z.shape[0], *z.shape[1:]), z.dtype) for z in zero_outs
    ]
    sh = NamedSharding(mesh, PartitionSpec("core"))
    dev_args = [jax.device_put(a, sh) for a in (concat_in + concat_zeros)]
    out_arrs = sharded(*dev_args)
    jax.block_until_ready(out_arrs)

    if timeit:
        ts = []
        for _ in range(timeit):
            t0 = time.perf_counter()
            r = sharded(*dev_args)
            jax.block_until_ready(r)
            ts.append(time.perf_counter() - t0)
        _timing["times"] = ts
        _timing["median_s"] = sorted(ts)[len(ts) // 2]
        # pipelined: issue NPIPE calls back-to-back, block once; amortizes
        # dispatch latency so per-call time approaches device exec time
        NPIPE = 24
        rs = []
        t0 = time.perf_counter()
        for _ in range(NPIPE):
            rs.append(sharded(*dev_args))
        jax.block_until_ready(rs)
        _timing["pipelined_s"] = (time.perf_counter() - t0) / NPIPE

    return [
        {
            nm: np.asarray(out_arrs[i]).reshape(n_cores, *out_avals[i].shape)[c]
            for i, nm in enumerate(out_names)
        }
        for c in range(n_cores)
    ]


def gcn_run(x, edge_index, edge_attr, bn1_g, bn1_b, W1, b1, bn2_g, bn2_b, W2, b2,
            n_cores=8, timeit=0):
    x = np.asarray(x, dtype=np.float32)
    n_nodes = x.shape[0]
    cores, K, nwin, nblk, P, nloc = _host_prep(edge_index, edge_attr, n_nodes, n_cores)

    nc = _build_nc(n_nodes, n_cores, K, nwin, nblk, P, nloc)

    common = dict(
        w1=np.asarray(W1, np.float32),
        w2=np.asarray(W2, np.float32),
        bn1g=np.asarray(bn1_g, np.float32).reshape(64, 1),
        bn1b=np.asarray(bn1_b, np.float32).reshape(64, 1),
        bn2g=np.asarray(bn2_g, np.float32).reshape(64, 1),
        bn2b=np.asarray(bn2_b, np.float32).reshape(64, 1),
        b1r=np.tile(np.asarray(b1, np.float32).reshape(1, 64), (128, 1)),
        b2r=np.tile(np.asarray(b2, np.float32).reshape(1, 64), (128, 1)),
        iotad=np.tile(np.arange(WIN, dtype=np.float16).reshape(1, WIN), (128, 1)),
        id128d=np.eye(128, dtype=np.float32),
        id64d=np.eye(64, dtype=np.float32),
    )
    in_maps = []
    for ci in range(n_cores):
        m = dict(common)
        m["x"] = x[ci * nloc : (ci + 1) * nloc]
        m["idxd"] = cores[ci]["idx16"]
        m["mcold"] = cores[ci]["mcol"]
        m["mwed"] = cores[ci]["mwe"]
        m["mwod"] = cores[ci]["mwo"]
        m["mwd"] = cores[ci]["mw"]
        in_maps.append(m)

    results = _run_spmd(nc, in_maps, n_cores, timeit=timeit)
    out = np.concatenate([results[ci]["out"] for ci in range(n_cores)], axis=0)
    return out


def kernel(**inputs):
    edge_index = np.asarray(inputs["edge_index"])
    timeit = int(os.environ.get("GCN_TIMEIT", "0"))
    out = gcn_run(
        inputs["x"], edge_index, inputs["edge_attr"],
        inputs["bn1_g"], inputs["bn1_b"], inputs["W1"], inputs["b1"],
        inputs["bn2_g"], inputs["bn2_b"], inputs["W2"], inputs["b2"],
        timeit=timeit,
    )
    return out, edge_index


# revision 19
# speedup vs baseline: 1.9080x; 1.9080x over previous
"""GCN block (BN -> lrelu -> GCNConv -> lrelu -> BN -> lrelu -> GCNConv -> lrelu)
on 8 Trainium2 NeuronCores via Bass/Tile.

Strategy: shard nodes (destinations) across cores; bucket edges by destination
window (64 dests) on the host; aggregate per window with one-hot fp16 matmuls
accumulating in fp32 PSUM; gather source features with dma_gather from an
AllGathered fp16 replica of h' = dinv * h; self-loops folded in analytically
in fp32. deg via the same one-hot matmuls against a ones vector. BN statistics
via matmul-with-ones partials + a tiny AllReduce.
"""

import os
import time

import numpy as np

EPS = 1e-5
SLOPE = 0.1
WIN = 64  # dests per window
GB = 16  # blocks per gather batch
AB = 16  # blocks per A-build batch

_timing = {}


class _StopBuild(Exception):
    pass


def _host_prep(edge_index, edge_attr, n_nodes, n_cores):
    """Bucket edges by (core, window); build uniform per-core slot arrays."""
    row = np.asarray(edge_index[0], dtype=np.int64)
    col = np.asarray(edge_index[1], dtype=np.int64)
    w = np.asarray(edge_attr[:, 0], dtype=np.float32)

    nloc = n_nodes // n_cores
    nwin = nloc // WIN

    order = np.argsort(col, kind="stable")
    row, col, w = row[order], col[order], w[order]

    gwin = col // WIN  # global window id: 0 .. n_cores*nwin-1
    counts = np.bincount(gwin, minlength=n_cores * nwin)
    K = int(np.ceil(counts.max() / 128)) if counts.max() > 0 else 1
    S = K * 128  # slots per window
    nblk = nwin * K  # blocks per core
    P = nwin * S  # slots per core

    offs = np.concatenate([[0], np.cumsum(counts)])
    rank = np.arange(len(row)) - offs[gwin]
    win_local = gwin % nwin
    core_of = gwin // nwin
    slot = win_local * S + rank  # within-core slot

    cores = []
    for ci in range(n_cores):
        m = core_of == ci
        er, ec, ew, es = row[m], col[m], w[m], slot[m]
        srow = np.zeros(P, dtype=np.int64)
        scol = np.full(P, 999.0, dtype=np.float16)  # sentinel: no match
        sw = np.zeros(P, dtype=np.float32)
        srow[es] = er
        scol[es] = (ec % WIN).astype(np.float16)
        sw[es] = ew
        parity = (srow & 1).astype(np.float32)
        pair = (srow >> 1).astype(np.int16)

        idx16 = np.ascontiguousarray(pair.reshape(P // 16, 16).T)  # [16, P/16]
        idx16 = np.tile(idx16, (8, 1))  # [128, P/16]
        mcol = np.ascontiguousarray(scol.reshape(nblk, 128).T)  # [128, nblk]
        swe = (sw * (1.0 - parity)).astype(np.float16)
        swo = (sw * parity).astype(np.float16)
        mwe = np.ascontiguousarray(swe.reshape(nblk, 128).T)
        mwo = np.ascontiguousarray(swo.reshape(nblk, 128).T)

        def dbl(a):  # [128, nblk] -> [128, nblk, 2] duplicated values
            return np.ascontiguousarray(np.repeat(a[:, :, None], 2, axis=2))

        mw = np.ascontiguousarray(sw.astype(np.float16).reshape(nblk, 128).T)
        cores.append(
            dict(idx16=idx16, mcol=dbl(mcol), mwe=dbl(mwe), mwo=dbl(mwo), mw=mw)
        )

    return cores, K, nwin, nblk, P, nloc


def _build_nc(n_nodes, n_cores, K, nwin, nblk, P, nloc, stop_after="all", no_cc=False):
    import concourse.bacc as bacc
    import concourse.mybir as mybir
    import concourse.tile as tile
    from concourse.library_config import mlp

    f32 = mybir.dt.float32
    f16 = mybir.dt.float16
    Alu = mybir.AluOpType

    npair = n_nodes // 2
    nch = nloc // 128  # node chunks per core (= window pairs)
    npr = nwin // 2
    assert npr == nch

    nc = bacc.Bacc(
        "TRN2", target_bir_lowering=False, debug=False, num_devices=n_cores
    )

    # --- I/O ---
    x = nc.dram_tensor("x", [nloc, 64], f32, kind="ExternalInput")
    w1 = nc.dram_tensor("w1", [64, 64], f32, kind="ExternalInput")
    w2 = nc.dram_tensor("w2", [64, 64], f32, kind="ExternalInput")
    bn1g = nc.dram_tensor("bn1g", [64, 1], f32, kind="ExternalInput")
    bn1b = nc.dram_tensor("bn1b", [64, 1], f32, kind="ExternalInput")
    bn2g = nc.dram_tensor("bn2g", [64, 1], f32, kind="ExternalInput")
    bn2b = nc.dram_tensor("bn2b", [64, 1], f32, kind="ExternalInput")
    b1r = nc.dram_tensor("b1r", [128, 64], f32, kind="ExternalInput")
    b2r = nc.dram_tensor("b2r", [128, 64], f32, kind="ExternalInput")
    idxd = nc.dram_tensor("idxd", [128, P // 16], mybir.dt.int16, kind="ExternalInput")
    iotad = nc.dram_tensor("iotad", [128, WIN], f16, kind="ExternalInput")
    id128d = nc.dram_tensor("id128d", [128, 128], f32, kind="ExternalInput")
    id64d = nc.dram_tensor("id64d", [64, 64], f32, kind="ExternalInput")
    mcold = nc.dram_tensor("mcold", [128, nblk, 2], f16, kind="ExternalInput")
    mwed = nc.dram_tensor("mwed", [128, nblk, 2], f16, kind="ExternalInput")
    mwod = nc.dram_tensor("mwod", [128, nblk, 2], f16, kind="ExternalInput")
    mwd = nc.dram_tensor("mwd", [128, nblk], f16, kind="ExternalInput")
    out = nc.dram_tensor("out", [nloc, 64], f32, kind="ExternalOutput")

    groups = [list(range(n_cores))]

    with tile.TileContext(nc) as tc:
      try:
        with (
            tc.tile_pool(name="pc", bufs=1) as pc,
            tc.tile_pool(name="pbig", bufs=1) as pbig,
            tc.tile_pool(name="pw", bufs=4) as pw,
            tc.tile_pool(name="pg", bufs=3) as pg,
            tc.tile_pool(name="ps", bufs=1, space="PSUM") as pstat,
            tc.tile_pool(name="pp", bufs=4, space="PSUM") as pagg,
            tc.tile_pool(name="pm", bufs=2, space="PSUM") as pmm,
            tc.tile_pool(name="dram", bufs=1, space="DRAM") as dram,
        ):
            nc.gpsimd.load_library(mlp)

            # ---------- constants & loads ----------
            idx_t = pc.tile([128, P // 16], mybir.dt.int16)
            nc.sync.dma_start(idx_t[:], idxd[:])
            mcol_t = pc.tile([128, nblk, 2], f16)
            nc.sync.dma_start(mcol_t[:], mcold[:])
            mwe_t = pc.tile([128, nblk, 2], f16)
            nc.sync.dma_start(mwe_t[:], mwed[:])
            mwo_t = pc.tile([128, nblk, 2], f16)
            nc.sync.dma_start(mwo_t[:], mwod[:])
            mw_t = pc.tile([128, nblk], f16)
            nc.sync.dma_start(mw_t[:], mwd[:])

            w1_t = pc.tile([64, 64], f32)
            nc.sync.dma_start(w1_t[:], w1[:])
            w2_t = pc.tile([64, 64], f32)
            nc.sync.dma_start(w2_t[:], w2[:])
            bn1g_t = pc.tile([64, 1], f32)
            nc.sync.dma_start(bn1g_t[:], bn1g[:])
            bn1b_t = pc.tile([64, 1], f32)
            nc.sync.dma_start(bn1b_t[:], bn1b[:])
            bn2g_t = pc.tile([64, 1], f32)
            nc.sync.dma_start(bn2g_t[:], bn2g[:])
            bn2b_t = pc.tile([64, 1], f32)
            nc.sync.dma_start(bn2b_t[:], bn2b[:])
            b1r_t = pc.tile([128, 1, 64], f32)
            nc.sync.dma_start(b1r_t[:, 0, :], b1r[:])
            b2r_t = pc.tile([128, 1, 64], f32)
            nc.sync.dma_start(b2r_t[:, 0, :], b2r[:])

            # big_a: x chunks early, reused as final z2 late
            big_a = pbig.tile([128, nch, 64], f32, tag="big_a")
            nc.sync.dma_start(big_a[:], x.rearrange("(c p) f -> p c f", p=128))

            iota_f = pc.tile([128, 1, WIN], f16)
            nc.sync.dma_start(iota_f[:, 0, :], iotad[:])
            id128 = pc.tile([128, 128], f32)
            nc.sync.dma_start(id128[:], id128d[:])
            id64 = pc.tile([64, 64], f32)
            nc.sync.dma_start(id64[:], id64d[:])
            ones16 = pc.tile([128, 1], f16)
            nc.vector.memset(ones16[:], 1.0)
            ones32 = pc.tile([128, 1], f32)
            nc.vector.memset(ones32[:], 1.0)

            # DRAM intermediates
            h1_full = dram.tile([npair, 128], f16, tag="h1f")
            h2_full = dram.tile([npair, 128], f16, tag="h2f")
            h1_b = dram.tile([nloc, 64], f16, tag="h1b")
            h2_b = dram.tile([nloc, 64], f16, tag="h2b")

            # ---------- helper: A-build for a batch of blocks ----------
            def build_A(b0, nb):
                # [128, nb, 32, 2] views: innermost step-1 pair over doubled
                # metadata keeps every AP 2x_1p-eligible on DVE (fp16 2x mode)
                g2 = WIN // 2
                eq = pw.tile([128, AB, g2, 2], f16, tag="eq")
                iot = iota_f[:, :, :].rearrange("p o (g t) -> p o g t", t=2)
                nc.vector.tensor_tensor(
                    out=eq[:, 0:nb, :, :],
                    in0=iot.to_broadcast([128, nb, g2, 2]),
                    in1=mcol_t[:, b0 : b0 + nb, None, :].to_broadcast(
                        [128, nb, g2, 2]
                    ),
                    op=Alu.is_equal,
                )
                ae = pw.tile([128, AB, g2, 2], f16, tag="ae")
                nc.vector.tensor_tensor(
                    out=ae[:, 0:nb, :, :],
                    in0=eq[:, 0:nb, :, :],
                    in1=mwe_t[:, b0 : b0 + nb, None, :].to_broadcast(
                        [128, nb, g2, 2]
                    ),
                    op=Alu.mult,
                )
                ao = pw.tile([128, AB, g2, 2], f16, tag="ao")
                nc.vector.tensor_tensor(
                    out=ao[:, 0:nb, :, :],
                    in0=eq[:, 0:nb, :, :],
                    in1=mwo_t[:, b0 : b0 + nb, None, :].to_broadcast(
                        [128, nb, g2, 2]
                    ),
                    op=Alu.mult,
                )
                return ae, ao

            # ---------- deg pass: deg = 1 + sum_w by dest (eq-only A, w as rhs)
            def build_eq(b0, nb):
                g2 = WIN // 2
                eq = pw.tile([128, AB, g2, 2], f16, tag="eq")
                iot = iota_f[:, :, :].rearrange("p o (g t) -> p o g t", t=2)
                nc.vector.tensor_tensor(
                    out=eq[:, 0:nb, :, :],
                    in0=iot.to_broadcast([128, nb, g2, 2]),
                    in1=mcol_t[:, b0 : b0 + nb, None, :].to_broadcast(
                        [128, nb, g2, 2]
                    ),
                    op=Alu.is_equal,
                )
                return eq

            deg_t = pc.tile([128, nch], f32)
            eq_d = None
            for j in range(npr):  # window pair j -> dests [128j, 128j+128)
                dps = pagg.tile([128, 64], f32, tag="agg")
                for half in range(2):
                    wnd = 2 * j + half
                    for k in range(K):
                        b = wnd * K + k
                        if b % AB == 0:
                            eq_d = build_eq(b, min(AB, nblk - b))
                        jj = b % AB
                        nc.tensor.matmul(
                            dps[64 * half : 64 * half + 64, 0:1],
                            lhsT=eq_d[:, jj, :, :],
                            rhs=mw_t[:, b : b + 1],
                            start=(k == 0),
                            stop=(k == K - 1),
                        )
                nc.vector.tensor_scalar(
                    deg_t[:, j : j + 1], dps[:, 0:1], 1.0, None, Alu.add
                )

            # dinv = rsqrt(deg): sqrt + reciprocal + one Newton step
            sqd_t = pw.tile([128, nch], f32, tag="nwt_s")
            nc.scalar.sqrt(sqd_t[:], deg_t[:])
            r0_t = pw.tile([128, nch], f32, tag="nwt_r")
            nc.vector.reciprocal(r0_t[:], sqd_t[:])
            a_t = pw.tile([128, nch], f32, tag="nwt_a")
            nc.vector.tensor_tensor(a_t[:], r0_t[:], r0_t[:], op=Alu.mult)
            nc.vector.tensor_tensor(a_t[:], deg_t[:], a_t[:], op=Alu.mult)
            nc.vector.tensor_scalar(a_t[:], a_t[:], -0.5, 1.5, Alu.mult, Alu.add)
            dinv_t = pc.tile([128, nch], f32)
            nc.vector.tensor_tensor(dinv_t[:], r0_t[:], a_t[:], op=Alu.mult)
            dinv2_t = pc.tile([128, nch], f32)
            nc.vector.tensor_tensor(dinv2_t[:], dinv_t[:], dinv_t[:], op=Alu.mult)

            # ---------- BN stats helper (node-major input chunks) ----------
            def bn_stats(src_all, tag):
                sps = pstat.tile([64, 1], f32, tag="ssum")
                spq = pstat.tile([64, 64], f32, tag="sq")
                for c in range(nch):
                    nc.tensor.matmul(
                        sps[:], lhsT=src_all[:, c, :], rhs=ones32[:],
                        start=(c == 0), stop=(c == nch - 1),
                    )
                for c in range(nch):
                    nc.tensor.matmul(
                        spq[:], lhsT=src_all[:, c, :], rhs=src_all[:, c, :],
                        start=(c == 0), stop=(c == nch - 1),
                    )
                st = pw.tile([64, 2], f32, tag="st")
                nc.vector.tensor_copy(st[:, 0:1], sps[:])
                dq = pw.tile([64, 64], f32, tag="dq")
                nc.vector.tensor_tensor(dq[:], spq[:], id64[:], op=Alu.mult)
                nc.vector.reduce_sum(st[:, 1:2], dq[:], axis=mybir.AxisListType.X)
                sin = dram.tile([64, 2], f32, tag=tag + "_in")
                sout = dram.tile([64, 2], f32, tag=tag + "_out")
                nc.gpsimd.dma_start(sin[:], st[:])
                if no_cc:
                    nc.gpsimd.dma_start(sout[:], sin[:])
                else:
                    nc.gpsimd.collective_compute(
                        "AllReduce", Alu.add, replica_groups=groups,
                        ins=[sin.opt()], outs=[sout.opt()],
                    )
                sr = pw.tile([64, 2], f32, tag="str")
                nc.gpsimd.dma_start(sr[:], sout[:])
                return sr

            def bn_coeffs(sr, g_t, b_t, tag):
                m = pw.tile([64, 1], f32, tag=tag + "m")
                nc.vector.tensor_scalar(m[:], sr[:, 0:1], 1.0 / n_nodes, None, Alu.mult)
                ms = pw.tile([64, 1], f32, tag=tag + "ms")
                nc.vector.tensor_scalar(ms[:], sr[:, 1:2], 1.0 / n_nodes, None, Alu.mult)
                v = pw.tile([64, 1], f32, tag=tag + "v")
                nc.vector.tensor_tensor(v[:], m[:], m[:], op=Alu.mult)
                nc.vector.tensor_tensor(v[:], ms[:], v[:], op=Alu.subtract)
                nc.vector.tensor_scalar(v[:], v[:], EPS, None, Alu.add)
                s = pw.tile([64, 1], f32, tag=tag + "s")
                nc.scalar.sqrt(s[:], v[:])
                r0 = pw.tile([64, 1], f32, tag=tag + "r0")
                nc.vector.reciprocal(r0[:], s[:])
                aa = pw.tile([64, 1], f32, tag=tag + "aa")
                nc.vector.tensor_tensor(aa[:], r0[:], r0[:], op=Alu.mult)
                nc.vector.tensor_tensor(aa[:], v[:], aa[:], op=Alu.mult)
                nc.vector.tensor_scalar(aa[:], aa[:], -0.5, 1.5, Alu.mult, Alu.add)
                r1 = pw.tile([64, 1], f32, tag=tag + "r1")
                nc.vector.tensor_tensor(r1[:], r0[:], aa[:], op=Alu.mult)
                scl = pw.tile([64, 1], f32, tag=tag + "scl")
                nc.vector.tensor_tensor(scl[:], r1[:], g_t[:], op=Alu.mult)
                sh = pw.tile([64, 1], f32, tag=tag + "sh")
                nc.vector.tensor_tensor(sh[:], m[:], scl[:], op=Alu.mult)
                nc.vector.tensor_tensor(sh[:], b_t[:], sh[:], op=Alu.subtract)
                return scl, sh

            # ---------- feature-major block: transpose, BN apply, lrelu, @W ----------
            def feat_block(src_all, scl, sh, w_t, hp_all, self_all):
                ft = pbig.tile([64, nloc], f32, tag="featT")
                for c in range(nch):
                    tp = pmm.tile([64, 128], f32, tag="mm")
                    nc.tensor.transpose(tp[:], src_all[:, c, :], id128[:])
                    nc.vector.tensor_copy(ft[:, 128 * c : 128 * (c + 1)], tp[:])
                nc.vector.tensor_scalar(ft[:], ft[:], scl[:], sh[:], Alu.mult, Alu.add)
                for c in range(nch):
                    sl = ft[:, 128 * c : 128 * (c + 1)]
                    lt = pw.tile([64, 128], f32, tag="lrT")
                    nc.scalar.mul(lt[:], sl, SLOPE)
                    nc.vector.tensor_tensor(sl, sl, lt[:], op=Alu.max)
                    hp = pmm.tile([128, 64], f32, tag="mm")
                    nc.tensor.matmul(hp[:], lhsT=sl, rhs=w_t[:], start=True, stop=True)
                    nc.scalar.activation(
                        hp_all[:, c, :], hp[:],
                        mybir.ActivationFunctionType.Copy,
                        bias=0.0, scale=dinv_t[:, c : c + 1],
                    )
                    nc.vector.tensor_scalar(
                        self_all[:, c, :], hp[:], dinv2_t[:, c : c + 1],
                        None, Alu.mult,
                    )

            if stop_after == "deg":
                nc.sync.dma_start(
                    out.rearrange("(c p) k -> p c k", p=128)[:, :, 0:1],
                    dinv_t[:, :, None],
                )
                raise _StopBuild
            # ---------- BN1 on x, h1' ----------
            sr1 = bn_stats(big_a, "st1")
            scl1, sh1 = bn_coeffs(sr1, bn1g_t, bn1b_t, "bn1")
            hp_all = pbig.tile([128, nch, 64], f16, tag="hp")
            self1 = pbig.tile([128, nch, 64], f32, tag="self1")
            feat_block(big_a, scl1, sh1, w1_t, hp_all, self1)
            if stop_after == "h1":
                nc.sync.dma_start(out.rearrange("(c p) f -> p c f", p=128), self1[:])
                raise _StopBuild
            nc.gpsimd.dma_start(h1_b.rearrange("(c p) f -> p c f", p=128), hp_all[:])
            if no_cc:
                for ci in range(n_cores):
                    nc.gpsimd.dma_start(
                        h1_full[ci * (nloc // 2) : (ci + 1) * (nloc // 2), :],
                        h1_b.rearrange("(a b) c -> a (b c)", b=2),
                    )
            else:
                nc.gpsimd.collective_compute(
                    "AllGather", Alu.bypass, replica_groups=groups,
                    ins=[h1_b.opt()], outs=[h1_full.opt()],
                )

            # ---------- aggregation pass ----------
            def agg_pass(h_full, z_all):
                ngb = (nblk + GB - 1) // GB
                ps = None
                ae_t = ao_t = None
                for g in range(ngb):
                    b0 = g * GB
                    nb = min(GB, nblk - b0)
                    mt = pg.tile([128, GB, 128], f16, tag="m")
                    nc.gpsimd.dma_gather(
                        out_ap=mt[:, 0:nb, :],
                        in_ap=h_full[:],
                        idxs_ap=idx_t[:, b0 * 8 : (b0 + nb) * 8],
                        num_idxs=nb * 128,
                        num_idxs_reg=nb * 128,
                        elem_size=128,
                        single_packet=False,
                    )
                    for b in range(b0, b0 + nb):
                        if b % AB == 0:
                            ae_t, ao_t = build_A(b, min(AB, nblk - b))
                        jj = b % AB
                        wnd = b // K
                        k = b % K
                        j = wnd // 2
                        half = wnd % 2
                        if half == 0 and k == 0:
                            ps = pagg.tile([128, 64], f32, tag="agg")
                        mj = b - b0
                        nc.tensor.matmul(
                            ps[64 * half : 64 * half + 64, :],
                            lhsT=ae_t[:, jj, :, :],
                            rhs=mt[:, mj, 0:64],
                            start=(k == 0),
                            stop=False,
                        )
                        nc.tensor.matmul(
                            ps[64 * half : 64 * half + 64, :],
                            lhsT=ao_t[:, jj, :, :],
                            rhs=mt[:, mj, 64:128],
                            start=False,
                            stop=(k == K - 1),
                        )
                        if half == 1 and k == K - 1:
                            nc.scalar.activation(
                                z_all[:, j, :], ps[:],
                                mybir.ActivationFunctionType.Copy,
                                bias=0.0, scale=dinv_t[:, j : j + 1],
                            )

            def post_agg(z_all, self_all, br_t):
                nc.vector.tensor_tensor(z_all[:], z_all[:], self_all[:], op=Alu.add)
                nc.vector.tensor_tensor(
                    z_all[:], z_all[:], br_t[:].to_broadcast([128, nch, 64]),
                    op=Alu.add,
                )
                for c in range(nch):
                    sl = z_all[:, c, :]
                    lt = pw.tile([128, 64], f32, tag="lrz")
                    nc.scalar.mul(lt[:], sl, SLOPE)
                    nc.vector.tensor_tensor(sl, sl, lt[:], op=Alu.max)

            if stop_after == "ag1":
                nc.sync.dma_start(out.rearrange("(c p) f -> p c f", p=128), self1[:])
                raise _StopBuild
            if stop_after == "aggnc":
                # fill h1_full locally (replicated shards) to test agg without AllGather
                for ci in range(n_cores):
                    nc.gpsimd.dma_start(
                        h1_full[ci * (nloc // 2) : (ci + 1) * (nloc // 2), :],
                        h1_b.rearrange("(a b) c -> a (b c)", b=2),
                    )
                z_all = pbig.tile([128, nch, 64], f32, tag="z_all")
                agg_pass(h1_full, z_all)
                nc.sync.dma_start(out.rearrange("(c p) f -> p c f", p=128), z_all[:])
                raise _StopBuild
            z_all = pbig.tile([128, nch, 64], f32, tag="z_all")
            agg_pass(h1_full, z_all)
            post_agg(z_all, self1, b1r_t)
            if stop_after == "agg1":
                nc.sync.dma_start(out.rearrange("(c p) f -> p c f", p=128), z_all[:])
                raise _StopBuild

            # ---------- BN2, h2' ----------
            sr2 = bn_stats(z_all, "st2")
            scl2, sh2 = bn_coeffs(sr2, bn2g_t, bn2b_t, "bn2")
            hp2_all = pbig.tile([128, nch, 64], f16, tag="hp")
            self2 = pbig.tile([128, nch, 64], f32, tag="self1")
            feat_block(z_all, scl2, sh2, w2_t, hp2_all, self2)
            if stop_after == "bn2":
                nc.sync.dma_start(out.rearrange("(c p) f -> p c f", p=128), self2[:])
                raise _StopBuild
            nc.gpsimd.dma_start(h2_b.rearrange("(c p) f -> p c f", p=128), hp2_all[:])
            if no_cc:
                for ci in range(n_cores):
                    nc.gpsimd.dma_start(
                        h2_full[ci * (nloc // 2) : (ci + 1) * (nloc // 2), :],
                        h2_b.rearrange("(a b) c -> a (b c)", b=2),
                    )
            else:
                nc.gpsimd.collective_compute(
                    "AllGather", Alu.bypass, replica_groups=groups,
                    ins=[h2_b.opt()], outs=[h2_full.opt()],
                )

            # ---------- layer 2 aggregation + final ----------
            z2_all = pbig.tile([128, nch, 64], f32, tag="big_a")
            agg_pass(h2_full, z2_all)
            post_agg(z2_all, self2, b2r_t)
            nc.sync.dma_start(out.rearrange("(c p) f -> p c f", p=128), z2_all[:])
      except _StopBuild:
        pass

    nc.compile()
    return nc


def _run_spmd(nc, in_maps, n_cores, timeit=0):
    """Run via PJRT/shard_map (axon path), optionally timing repeated execs."""
    import jax
    from jax.sharding import Mesh, NamedSharding, PartitionSpec
    from jax.experimental.shard_map import shard_map
    import concourse.mybir as mybir
    from concourse import bass2jax

    bass2jax.install_neuronx_cc_hook()

    partition_name = nc.partition_id_tensor.name if nc.partition_id_tensor else None
    in_names, out_names, out_avals, zero_outs = [], [], [], []
    for alloc in nc.m.functions[0].allocations:
        if not isinstance(alloc, mybir.MemoryLocationSet):
            continue
        name = alloc.memorylocations[0].name
        if alloc.kind == "ExternalInput":
            if name != partition_name:
                in_names.append(name)
        elif alloc.kind == "ExternalOutput":
            shape = tuple(alloc.tensor_shape)
            dtype = mybir.dt.np(alloc.dtype)
            out_names.append(name)
            out_avals.append(jax.core.ShapedArray(shape, dtype))
            zero_outs.append(np.zeros(shape, dtype))
    n_params = len(in_names)
    all_in_names = in_names + out_names + ([partition_name] if partition_name else [])

    def _body(*args):
        operands = list(args)
        if partition_name is not None:
            operands.append(bass2jax.partition_id_tensor())
        outs = bass2jax._bass_exec_p.bind(
            *operands,
            out_avals=tuple(out_avals),
            in_names=tuple(all_in_names),
            out_names=tuple(out_names),
            lowering_input_output_aliases=(),
            sim_require_finite=True,
            sim_require_nnan=True,
            nc=nc,
        )
        return tuple(outs)

    devices = jax.devices()[:n_cores]
    mesh = Mesh(np.asarray(devices), ("core",))
    in_specs = (PartitionSpec("core"),) * (n_params + len(out_names))
    out_specs = (PartitionSpec("core"),) * len(out_names)
    sharded = jax.jit(
        shard_map(_body, mesh=mesh, in_specs=in_specs, out_specs=out_specs,
                  check_rep=False),
        keep_unused=True,
    )
    concat_in = [
        np.concatenate([np.asarray(in_maps[c][nm]) for c in range(n_cores)], axis=0)
        for nm in in_names
    ]
    concat_zeros = [
        np.zeros((n_cores * z.shape[0], *z.shape[1:]), z.dtype) for z in zero_outs
    ]
    sh = NamedSharding(mesh, PartitionSpec("core"))
    dev_args = [jax.device_put(a, sh) for a in (concat_in + concat_zeros)]
    out_arrs = sharded(*dev_args)
    jax.block_until_ready(out_arrs)

    if timeit:
        ts = []
        for _ in range(timeit):
            t0 = time.perf_counter()
            r = sharded(*dev_args)
            jax.block_until_ready(r)
            ts.append(time.perf_counter() - t0)
        _timing["times"] = ts
        _timing["median_s"] = sorted(ts)[len(ts) // 2]
        # pipelined: issue NPIPE calls back-to-back, block once; amortizes
        # dispatch latency so per-call time approaches device exec time
        NPIPE = 24
        rs = []
        t0 = time.perf_counter()
        for _ in range(NPIPE):
            rs.append(sharded(*dev_args))
        jax.block_until_ready(rs)
        _timing["pipelined_s"] = (time.perf_counter() - t0) / NPIPE

    return [
        {
            nm: np.asarray(out_arrs[i]).reshape(n_cores, *out_avals[i].shape)[c]
            for i, nm in enumerate(out_names)
        }
        for c in range(n_cores)
    ]


def gcn_run(x, edge_index, edge_attr, bn1_g, bn1_b, W1, b1, bn2_g, bn2_b, W2, b2,
            n_cores=8, timeit=0):
    x = np.asarray(x, dtype=np.float32)
    n_nodes = x.shape[0]
    cores, K, nwin, nblk, P, nloc = _host_prep(edge_index, edge_attr, n_nodes, n_cores)

    nc = _build_nc(n_nodes, n_cores, K, nwin, nblk, P, nloc)

    common = dict(
        w1=np.asarray(W1, np.float32),
        w2=np.asarray(W2, np.float32),
        bn1g=np.asarray(bn1_g, np.float32).reshape(64, 1),
        bn1b=np.asarray(bn1_b, np.float32).reshape(64, 1),
        bn2g=np.asarray(bn2_g, np.float32).reshape(64, 1),
        bn2b=np.asarray(bn2_b, np.float32).reshape(64, 1),
        b1r=np.tile(np.asarray(b1, np.float32).reshape(1, 64), (128, 1)),
        b2r=np.tile(np.asarray(b2, np.float32).reshape(1, 64), (128, 1)),
        iotad=np.tile(np.arange(WIN, dtype=np.float16).reshape(1, WIN), (128, 1)),
        id128d=np.eye(128, dtype=np.float32),
        id64d=np.eye(64, dtype=np.float32),
    )
    in_maps = []
    for ci in range(n_cores):
        m = dict(common)
        m["x"] = x[ci * nloc : (ci + 1) * nloc]
        m["idxd"] = cores[ci]["idx16"]
        m["mcold"] = cores[ci]["mcol"]
        m["mwed"] = cores[ci]["mwe"]
        m["mwod"] = cores[ci]["mwo"]
        m["mwd"] = cores[ci]["mw"]
        in_maps.append(m)

    results = _run_spmd(nc, in_maps, n_cores, timeit=timeit)
    out = np.concatenate([results[ci]["out"] for ci in range(n_cores)], axis=0)
    return out


def kernel(**inputs):
    edge_index = np.asarray(inputs["edge_index"])
    timeit = int(os.environ.get("GCN_TIMEIT", "0"))
    out = gcn_run(
        inputs["x"], edge_index, inputs["edge_attr"],
        inputs["bn1_g"], inputs["bn1_b"], inputs["W1"], inputs["b1"],
        inputs["bn2_g"], inputs["bn2_b"], inputs["W2"], inputs["b2"],
        timeit=timeit,
    )
    return out, edge_index
